# revision 2
# baseline (speedup 1.0000x reference)
"""Trainium2 Bass kernel for the conv-projected self-attention block.

Reference computation (B=8, C=256, N=64, K=256):
    q = wq @ x + bq; k = wk @ x + bk; v = wv @ x + bv      (1x1 convs over C)
    s = einsum('bcnk,bcnl->bnkl', q, k) / sqrt(C)
    p = softmax(s, axis=-1)
    o = einsum('bnkl,bcnl->bcnk', p, v)
    out = x + (wp @ o + bp)

Sharding: data-parallel over B — one batch per NeuronCore (8 cores).
All matmuls run in fp32r (full PE rate at free-dim>=256, ~fp32 accuracy).

Per-core dataflow (batch b), per group of G n-slices:
  load X[ci-tile, G*256]                                   (2 DMAs)
  Q = wqT.T @ X + bq          -> [c, pos]   (DVE evict)
  K = wkT.T @ X + bk          -> [c, pos]   (ACT evict)
  Vt = X.T @ wvT  (no bias)   -> [pos, c]   (ACT evict)  # bv folded into bp'
  per slice s:
    St = K.T @ Q              -> [l, kk] PSUM
    Ep = exp(St/16)           -> SBUF f32r (ACT, no max-subtraction needed)
    Sum = ones.T @ Ep         -> [128, kk] PSUM (sum replicated on partitions)
    R = 1/Sum                 (DVE reciprocal)
    Att = (Vt.T @ Ep) * R     -> [c, kk]   (DVE tensor_tensor evict)
    Out = wpT.T @ Att + bp' + x  (DVE scalar_tensor_tensor evict)
    store Out                 (2 DMAs)
"""

import numpy as np

import concourse.bass as bass
import concourse.bacc as bacc
import concourse.mybir as mybir
import concourse.tile as tile
from concourse.bass_utils import run_bass_kernel_spmd

F32 = mybir.dt.float32
F32R = mybir.dt.float32r
AF = mybir.ActivationFunctionType
ALU = mybir.AluOpType

B, C, N, K = 8, 256, 64, 256
G = 4                 # n-slices per group
NG = N // G           # number of groups
SCALE = 1.0 / 16.0    # 1/sqrt(C)

_CACHE = {}


def _build():
    nc = bacc.Bacc("TRN2", target_bir_lowering=False, debug=False, num_devices=8)

    x_d = nc.dram_tensor("x", [C, N, K], F32R, kind="ExternalInput")
    wqt_d = nc.dram_tensor("wqt", [C, C], F32R, kind="ExternalInput")
    wkt_d = nc.dram_tensor("wkt", [C, C], F32R, kind="ExternalInput")
    wvt_d = nc.dram_tensor("wvt", [C, C], F32R, kind="ExternalInput")
    wpt_d = nc.dram_tensor("wpt", [C, C], F32R, kind="ExternalInput")
    bq_d = nc.dram_tensor("bq", [C], F32, kind="ExternalInput")
    bk_d = nc.dram_tensor("bk", [C], F32, kind="ExternalInput")
    bpe_d = nc.dram_tensor("bpe", [C], F32, kind="ExternalInput")
    y_d = nc.dram_tensor("y", [C, N, K], F32, kind="ExternalOutput")

    with tile.TileContext(nc) as tc:
        with tc.tile_pool(name="const", bufs=1) as const, \
             tc.tile_pool(name="xg", bufs=2) as xgp, \
             tc.tile_pool(name="qk", bufs=2) as qkp, \
             tc.tile_pool(name="vt", bufs=2) as vtp, \
             tc.tile_pool(name="sm", bufs=3) as smp, \
             tc.tile_pool(name="ot", bufs=3) as otp, \
             tc.tile_pool(name="ps_proj", bufs=2, space="PSUM") as ps_proj, \
             tc.tile_pool(name="ps_vs", bufs=3, space="PSUM") as ps_vs, \
             tc.tile_pool(name="ps_af", bufs=3, space="PSUM") as ps_af:

            # ---- constants ----
            wqt = const.tile([128, 2, C], F32R, name="wqt_s")   # dim1 = ci tile
            wkt = const.tile([128, 2, C], F32R, name="wkt_s")
            wvt = const.tile([128, 2, C], F32R, name="wvt_s")
            wpt = const.tile([128, 2, C], F32R, name="wpt_s")
            for ci in range(2):
                nc.sync.dma_start(out=wqt[:, ci, :], in_=wqt_d[bass.ts(ci, 128), :])
                nc.sync.dma_start(out=wkt[:, ci, :], in_=wkt_d[bass.ts(ci, 128), :])
                nc.sync.dma_start(out=wvt[:, ci, :], in_=wvt_d[bass.ts(ci, 128), :])
                nc.sync.dma_start(out=wpt[:, ci, :], in_=wpt_d[bass.ts(ci, 128), :])
            bq_s = const.tile([128, 2], F32, name="bq_s")
            bk_s = const.tile([128, 2], F32, name="bk_s")
            bpe_s = const.tile([128, 2], F32, name="bpe_s")
            for t in range(2):
                nc.sync.dma_start(out=bq_s[:, t:t + 1], in_=bq_d[bass.ts(t, 128)])
                nc.sync.dma_start(out=bk_s[:, t:t + 1], in_=bk_d[bass.ts(t, 128)])
                nc.sync.dma_start(out=bpe_s[:, t:t + 1], in_=bpe_d[bass.ts(t, 128)])
            ones_f = const.tile([128, 128], F32, name="ones_f")
            nc.vector.memset(ones_f, 1.0)
            ones = const.tile([128, 128], F32R, name="ones_s")
            nc.vector.tensor_copy(ones, ones_f)

            for g in range(NG):
                n0 = g * G
                # ---- load X group: [128, G, 256] per ci tile ----
                xg = []
                for ci in range(2):
                    xt = xgp.tile([128, G, K], F32R, name="xt", tag=f"x{ci}")
                    nc.sync.dma_start(
                        out=xt, in_=x_d[bass.ts(ci, 128), n0:n0 + G, :])
                    xg.append(xt)

                # ---- Q/K projections: [128, 2(co), G, 256] ----
                qg = qkp.tile([128, 2, G, K], F32R, name="qg", tag="qg")
                kg = qkp.tile([128, 2, G, K], F32R, name="kg", tag="kg")
                nch = (G * K) // 512  # 512-wide chunks per group
                for co in range(2):
                    for ch in range(nch):
                        csl = slice(2 * ch, 2 * ch + 2)  # two 256-slices
                        psq = ps_proj.tile([128, 512], F32, name="psq", tag="proj")
                        for ci in range(2):
                            nc.tensor.matmul(
                                psq, wqt[:, ci, bass.ts(co, 128)],
                                xg[ci][:, csl, :].rearrange("p a b -> p (a b)"),
                                start=(ci == 0), stop=(ci == 1))
                        nc.vector.tensor_scalar_add(
                            qg[:, co, csl, :].rearrange("p a b -> p (a b)"),
                            psq, bq_s[:, co:co + 1])
                        psk = ps_proj.tile([128, 512], F32, name="psk", tag="proj")
                        for ci in range(2):
                            nc.tensor.matmul(
                                psk, wkt[:, ci, bass.ts(co, 128)],
                                xg[ci][:, csl, :].rearrange("p a b -> p (a b)"),
                                start=(ci == 0), stop=(ci == 1))
                        nc.scalar.activation(
                            out=kg[:, co, csl, :].rearrange("p a b -> p (a b)"),
                            in_=psk, func=AF.Identity, bias=bk_s[:, co:co + 1])

                # ---- Vt projection: [128, 2G(pos tile), 256(c)] ----
                vt = vtp.tile([128, 2 * G, C], F32R, name="vt", tag="vt")
                for pt in range(2 * G):
                    psv = ps_vs.tile([128, C], F32, name="psv", tag="vs")
                    for ci in range(2):
                        nc.tensor.matmul(
                            psv,
                            xg[ci][:, pt // 2, bass.ts(pt % 2, 128)],
                            wvt[:, ci, :],
                            start=(ci == 0), stop=(ci == 1))
                    nc.scalar.copy(vt[:, pt, :], psv)

                # ---- attention per slice ----
                for s in range(G):
                    n_abs = n0 + s
                    # St = K^T Q -> [l(2x128), kk=256]; Ep = exp(St/16)
                    ep = smp.tile([128, 2, K], F32R, name="ep", tag="ep")
                    for lt in range(2):
                        pss = ps_vs.tile([128, K], F32, name="pss", tag="vs")
                        for ct in range(2):
                            nc.tensor.matmul(
                                pss,
                                kg[:, ct, s, bass.ts(lt, 128)],
                                qg[:, ct, s, :],
                                start=(ct == 0), stop=(ct == 1))
                        nc.scalar.activation(
                            out=ep[:, lt, :], in_=pss, func=AF.Exp, scale=SCALE)

                    # Sum over l, replicated across partitions; R = 1/Sum
                    psu = ps_vs.tile([128, K], F32, name="psu", tag="vs")
                    for lt in range(2):
                        nc.tensor.matmul(
                            psu, ones, ep[:, lt, :],
                            start=(lt == 0), stop=(lt == 1))
                    recip = smp.tile([128, K], F32, name="recip", tag="recip")
                    nc.vector.reciprocal(recip, psu)

                    # Att = (Vt.T @ Ep) * R -> [c(2x128), kk]
                    att = smp.tile([128, 2, K], F32R, name="att", tag="att")
                    for ct in range(2):
                        psa = ps_af.tile([128, K], F32, name="psa", tag="af")
                        for lt in range(2):
                            nc.tensor.matmul(
                                psa,
                                vt[:, 2 * s + lt, bass.ts(ct, 128)],
                                ep[:, lt, :],
                                start=(lt == 0), stop=(lt == 1))
                        nc.vector.tensor_tensor(
                            out=att[:, ct, :], in0=psa, in1=recip, op=ALU.mult)

                    # Out = wpT.T @ Att + bp' + x ; store
                    outf = otp.tile([128, 2, K], F32, name="outf", tag="outf")
                    for ot in range(2):
                        psf = ps_af.tile([128, K], F32, name="psf", tag="af")
                        for ct in range(2):
                            nc.tensor.matmul(
                                psf,
                                wpt[:, ct, bass.ts(ot, 128)],
                                att[:, ct, :],
                                start=(ct == 0), stop=(ct == 1))
                        nc.vector.scalar_tensor_tensor(
                            out=outf[:, ot, :], in0=psf,
                            scalar=bpe_s[:, ot:ot + 1],
                            in1=xg[ot][:, s, :].bitcast(F32),
                            op0=ALU.add, op1=ALU.add)
                        nc.sync.dma_start(
                            out=y_d[bass.ts(ot, 128), n_abs, :],
                            in_=outf[:, ot, :])

    nc.compile()
    return nc


def _get_nc():
    if "nc" not in _CACHE:
        _CACHE["nc"] = _build()
    return _CACHE["nc"]


def run(inputs, trace=False):
    x = np.ascontiguousarray(inputs["x"], dtype=np.float32)
    wq = inputs["wq"].astype(np.float32)
    wk = inputs["wk"].astype(np.float32)
    wv = inputs["wv"].astype(np.float32)
    wp = inputs["wp"].astype(np.float32)
    bq = inputs["bq"].astype(np.float32)
    bk = inputs["bk"].astype(np.float32)
    bv = inputs["bv"].astype(np.float32)
    bp = inputs["bp"].astype(np.float32)

    wqt = np.ascontiguousarray(wq.T)
    wkt = np.ascontiguousarray(wk.T)
    wvt = np.ascontiguousarray(wv.T)
    wpt = np.ascontiguousarray(wp.T)
    # v-bias folds through attention (rows of prob sum to 1) into the final
    # projection bias: out = wp @ (att + bv) + bp = wp @ att + (bp + wp @ bv)
    bpe = (bp.astype(np.float64) + wp.astype(np.float64) @ bv.astype(np.float64)
           ).astype(np.float32)

    nc = _get_nc()
    common = {"wqt": wqt, "wkt": wkt, "wvt": wvt, "wpt": wpt,
              "bq": bq, "bk": bk, "bpe": bpe}
    in_maps = [dict(common, x=x[b]) for b in range(B)]
    res = run_bass_kernel_spmd(nc, in_maps, core_ids=list(range(8)), trace=trace)
    out = np.stack([res.results[b]["y"] for b in range(B)], axis=0)
    return out, res


def kernel(**inputs):
    out, _ = run(inputs, trace=False)
    return out
